# revision 23
# baseline (speedup 1.0000x reference)
"""Trainium2 Bass kernel for Encoder+RegLSTM (embedding lookup -> LSTM -> mask).

Strategy (data-parallel over batch, 8 cores x 8 sequences):
  - The reference's sort-by-length + unsort is an exact identity (the LSTM is
    elementwise in batch), so it is skipped.
  - Embedding gather via SWDGE dma_gather(transpose=True) from a bf16 copy of
    the table, landing directly in x^T layout [E(part), token].
  - Input projection (x @ W_ih^T + b) computed chunk-wise as bf16 matmuls
    accumulating into PSUM (gates^T layout: gate-dim on partitions).
  - LSTM recurrence: per step, 16 weight-stationary bf16 matmuls accumulate
    W_hh @ h_{t-1} on top of the preloaded input projection in PSUM; gate
    nonlinearities on ScalarE, cell/hidden update on VectorE.
  - h history written to DRAM fp32 in its native on-chip layout
    [chunk, h-dim, (t, half, b)]; the host de-permutes while unsharding.
"""

import os
import sys

os.environ.setdefault("TILE_EXHAUSTIVE_MEMORY_SHARE_CHECK", "1")
sys.path.insert(0, "/opt/trn_rl_repo")

import numpy as np
import ml_dtypes

import concourse.tile as tile
from concourse import bacc, mybir, library_config
from concourse import bass_utils

BF16 = mybir.dt.bfloat16
F32 = mybir.dt.float32
I16 = mybir.dt.int16

V, E, H = 32000, 256, 256
G4 = 4 * H  # 1024 gate dims, 8 m-tiles of 128
NCORES = 8
BL = 8  # batch per core
S_FULL = 2048
PCHUNK = 32  # steps per PSUM chunk (4 banks)
GS = 64  # steps per gather chunk (dma_gather breaks somewhere in 512<n_idx<=1024)

# psum m-position gate map after host-side row permutation: i=(0,1) f=(2,3)
# o=(4,5) g=(6,7).  Emission order: g first (its Tanh overlaps the MM phase),
# o last (the merged Sigmoid over i,f,o starts when the phase ends).
MM_ORDER = [6, 7, 0, 1, 2, 3, 4, 5]


def build_nc(S=S_FULL, gs=GS):
    NP = S // PCHUNK
    NG = S // gs
    NIDX = gs * BL  # indices per gather chunk
    SPG = gs // PCHUNK  # pchunks per gather chunk

    nc = bacc.Bacc("TRN2", target_bir_lowering=False, debug=False)

    idx_d = nc.dram_tensor("idx", [NG, 128, NIDX // 16], I16, kind="ExternalInput")
    emb_d = nc.dram_tensor("embed", [V, E], BF16, kind="ExternalInput")
    wih_d = nc.dram_tensor("wih", [2, 128, G4], BF16, kind="ExternalInput")
    whh_d = nc.dram_tensor("whh", [2, 128, G4], BF16, kind="ExternalInput")
    bias_d = nc.dram_tensor("bias", [1, G4], BF16, kind="ExternalInput")
    out_d = nc.dram_tensor("out", [S // PCHUNK, 128, PCHUNK * 16], F32, kind="ExternalOutput")

    with tile.TileContext(nc) as tc:
        wpool = tc.alloc_tile_pool(name="w", bufs=1)
        ipool = tc.alloc_tile_pool(name="ip", bufs=2)
        xpool = tc.alloc_tile_pool(name="xp", bufs=2)
        pspool = tc.alloc_tile_pool(name="ps", bufs=2, space="PSUM")
        hpool = tc.alloc_tile_pool(name="hp", bufs=4)
        cpool = tc.alloc_tile_pool(name="cp", bufs=4)
        tpool = tc.alloc_tile_pool(name="tp", bufs=6)
        histpool = tc.alloc_tile_pool(name="hist", bufs=2)

        nc.gpsimd.load_library(library_config.mlp)

        # --- weights to SBUF ---
        wih = wpool.tile([128, 2, G4], BF16)
        nc.sync.dma_start(out=wih[:, :, :], in_=wih_d.ap().rearrange("c p n -> p c n"))
        whh = wpool.tile([128, 2, G4], BF16)
        nc.sync.dma_start(out=whh[:, :, :], in_=whh_d.ap().rearrange("c p n -> p c n"))
        bias = wpool.tile([1, G4], BF16)
        nc.sync.dma_start(out=bias[:, :], in_=bias_d[:, :])
        ones = wpool.tile([1, 256], BF16)
        nc.vector.memset(ones[:, :], 1.0)

        # --- initial state ---
        hT = hpool.tile([128, 2 * BL], BF16, tag="h")
        nc.vector.memset(hT[:, :], 0.0)
        cS0 = cpool.tile([128, 2 * BL], F32, tag="c")
        nc.vector.memset(cS0[:, :], 0.0)
        cS = cS0[:, :].rearrange("p (c b) -> p c b", c=2)

        xts = [None] * NG
        pss = [None] * NP

        def emit_gather(g):
            idx_sb = ipool.tile([128, NIDX // 16], I16)
            nc.sync.dma_start(out=idx_sb[:, :], in_=idx_d[g, :, :])
            xts[g] = xpool.tile([128, 2, NIDX], BF16, tag="xt", name="xt")
            nc.gpsimd.dma_gather(
                xts[g][:, :, :],
                emb_d[:, :],
                idx_sb[:, :],
                NIDX,
                NIDX,
                E,
                transpose=True,
            )

        # preGEMM piece list: per bank, even-m c0 (start=True) first
        PIECES = []
        for bk in range(4):
            me, mo = 2 * bk, 2 * bk + 1
            PIECES += [(me, 0, True), (mo, 0, False), (me, 1, False), (mo, 1, False)]
        PIECES += [(m, "bias", False) for m in range(8)]

        def emit_pregemm_piece(pc, i):
            if i == 0:
                pss[pc] = pspool.tile([128, PCHUNK * 64], F32, tag="psc", name="psc")
            ps = pss[pc]
            g, t0 = (pc * PCHUNK) // gs, (pc * PCHUNK) % gs
            m, c, st = PIECES[i]
            if c == "bias":
                return nc.tensor.matmul(
                    ps[:, m * 256 : (m + 1) * 256],
                    bias[0:1, m * 128 : (m + 1) * 128],
                    ones[0:1, 0:256],
                    start=False,
                    stop=False,
                    skip_group_check=True,
                )
            else:
                return nc.tensor.matmul(
                    ps[:, m * 256 : (m + 1) * 256],
                    wih[:, c, m * 128 : (m + 1) * 128],
                    xts[g][:, c, t0 * BL : (t0 + PCHUNK) * BL],
                    start=st,
                    stop=False,
                    skip_group_check=True,
                )

        def emit_step(t, hist):
            nonlocal hT, cS
            pc, tl = t // PCHUNK, t % PCHUNK
            ps = pss[pc]
            for c2 in range(2):
                for m in MM_ORDER:
                    nc.tensor.matmul(
                        ps[:, m * 256 + tl * BL : m * 256 + (tl + 1) * BL],
                        whh[:, c2, m * 128 : (m + 1) * 128],
                        hT[:, c2 * BL : (c2 + 1) * BL],
                        start=False,
                        stop=(m == 5 and c2 == 1),
                        skip_group_check=True,
                    )
            psv = ps[:, :].rearrange("p (m t b) -> p m t b", m=8, t=PCHUNK)
            tg = tpool.tile([128, 2 * BL], F32, tag="tg")
            nc.scalar.activation(
                tg[:, :].rearrange("p (c b) -> p c b", c=2),
                psv[:, 6:8, tl, :],
                mybir.ActivationFunctionType.Tanh,
            )
            sifo = tpool.tile([128, 6 * BL], F32, tag="sifo")
            nc.scalar.activation(
                sifo[:, :].rearrange("p (c b) -> p c b", c=6),
                psv[:, 0:6, tl, :],
                mybir.ActivationFunctionType.Sigmoid,
            )

            t1 = tpool.tile([128, 2 * BL], F32, tag="t1")
            nc.vector.tensor_mul(t1[:, :], sifo[:, 0 : 2 * BL], tg[:, :])
            t2 = tpool.tile([128, 2 * BL], F32, tag="t2")
            nc.vector.tensor_mul(
                t2[:, :].rearrange("p (c b) -> p c b", c=2),
                sifo[:, 2 * BL : 4 * BL].rearrange("p (c b) -> p c b", c=2),
                cS,
            )
            # store c in the dead g-gate psum slice of this step: Tanh then
            # reads PSUM (~75ns faster than an SBUF-source ACT op)
            cN = psv[:, 6:8, tl, :]
            cn_inst = nc.vector.tensor_add(
                cN,
                t1[:, :].rearrange("p (c b) -> p c b", c=2),
                t2[:, :].rearrange("p (c b) -> p c b", c=2),
            )
            tc_ = tpool.tile([128, 2 * BL], F32, tag="tc")
            nc.scalar.activation(
                tc_[:, :].rearrange("p (c b) -> p c b", c=2),
                cN,
                mybir.ActivationFunctionType.Tanh,
            )
            hN = hpool.tile([128, 2 * BL], BF16, tag="h")
            nc.vector.tensor_mul(hN[:, 0:BL], sifo[:, 4 * BL : 5 * BL], tc_[:, 0:BL])
            nc.vector.tensor_mul(hN[:, BL : 2 * BL], sifo[:, 5 * BL : 6 * BL], tc_[:, BL : 2 * BL])
            nc.vector.tensor_mul(
                hist[:, tl * 16 : (tl + 1) * 16], sifo[:, 4 * BL : 6 * BL], tc_[:, :]
            )
            hT, cS = hN, cN
            return cn_inst

        def emit_out(pc, hist):
            nc.sync.dma_start(out=out_d[pc, :, :], in_=hist[:, :])

        emit_gather(0)
        for i in range(len(PIECES)):
            emit_pregemm_piece(0, i)
        for pc in range(NP):
            if pc % SPG == 0 and pc // SPG + 1 < NG:
                emit_gather(pc // SPG + 1)
            hist = histpool.tile([128, PCHUNK * 16], F32)
            for s in range(PCHUNK):
                cn_inst = emit_step(pc * PCHUNK + s, hist)
                if pc + 1 < NP and s < len(PIECES):
                    pinst = emit_pregemm_piece(pc + 1, s)
                    tile.add_dep_helper(pinst.ins, cn_inst.ins, sync=True, reason="pe warmup")
            emit_out(pc, hist)

        for p in (histpool, tpool, cpool, hpool, pspool, xpool, ipool, wpool):
            p.release()

    nc.compile()
    return nc


def make_inputs(text_inputs, embed, W_ih, W_hh, b_ih, b_hh, S=S_FULL, gs=GS):
    """Host-side marshaling into per-core in_maps."""
    NG = S // gs
    NIDX = gs * BL
    tok = np.asarray(text_inputs).astype(np.int32)
    emb_bf = np.asarray(embed).astype(ml_dtypes.bfloat16)
    # permute gate rows [i, f, g, o] -> [i, f, o, g] (kernel's psum layout)
    perm = np.concatenate([np.arange(0, 512), np.arange(768, 1024), np.arange(512, 768)])
    W_ih = np.asarray(W_ih)[perm]
    W_hh = np.asarray(W_hh)[perm]
    bsum = (np.asarray(b_ih) + np.asarray(b_hh))[perm]
    wih_t = np.ascontiguousarray(W_ih.T).reshape(2, 128, G4).astype(ml_dtypes.bfloat16)
    whh_t = np.ascontiguousarray(W_hh.T).reshape(2, 128, G4).astype(ml_dtypes.bfloat16)
    bias = bsum.reshape(1, G4).astype(ml_dtypes.bfloat16)

    in_maps = []
    for m in range(NCORES):
        tc_ = tok[m * BL : (m + 1) * BL, :S]
        idx = np.empty((NG, 128, NIDX // 16), np.int16)
        for g in range(NG):
            flat = tc_[:, g * gs : (g + 1) * gs].T.reshape(-1)  # (t, b) order
            wrapped = flat.reshape(-1, 16).T.astype(np.int16)  # [16, NIDX//16]
            idx[g] = np.tile(wrapped, (8, 1))
        in_maps.append(
            {"idx": idx, "embed": emb_bf, "wih": wih_t, "whh": whh_t, "bias": bias}
        )
    return in_maps


def unpermute_out(raw):
    """[NP, 128, PCHUNK*16] (ch, p, (tl, c, b)) -> [BL, S, 256]"""
    NP = raw.shape[0]
    v = raw.reshape(NP, 128, PCHUNK, 2, BL)  # ch, p, tl, c, b
    v = v.transpose(4, 0, 2, 3, 1)  # b, ch, tl, c, p
    return np.ascontiguousarray(v).reshape(BL, NP * PCHUNK, 2 * 128)


_nc_cache = {}


def _get_nc(S=S_FULL, gs=GS):
    key = (S, gs)
    if key not in _nc_cache:
        _nc_cache[key] = build_nc(S, gs)
    return _nc_cache[key]


def kernel(text_inputs, mask_input, len_seq, embed, W_ih, W_hh, b_ih, b_hh):
    nc = _get_nc()
    in_maps = make_inputs(text_inputs, embed, W_ih, W_hh, b_ih, b_hh)
    try:
        res = bass_utils.run_bass_kernel_spmd(nc, in_maps, core_ids=list(range(NCORES)))
    except Exception:
        # transient device-state failures recover on retry
        res = bass_utils.run_bass_kernel_spmd(nc, in_maps, core_ids=list(range(NCORES)))
    out = np.concatenate(
        [unpermute_out(res.results[m]["out"]) for m in range(NCORES)], axis=0
    )
    mask = np.asarray(mask_input)
    if not np.all(mask == 1.0):
        out = out * mask[..., None]
    return out.astype(np.float32)


# revision 24
# speedup vs baseline: 1.0007x; 1.0007x over previous
"""Trainium2 Bass kernel for Encoder+RegLSTM (embedding lookup -> LSTM -> mask).

Strategy (data-parallel over batch, 8 cores x 8 sequences):
  - The reference's sort-by-length + unsort is an exact identity (the LSTM is
    elementwise in batch), so it is skipped.
  - Embedding gather via SWDGE dma_gather(transpose=True) from a bf16 copy of
    the table, landing directly in x^T layout [E(part), token].
  - Input projection (x @ W_ih^T + b) computed chunk-wise as bf16 matmuls
    accumulating into PSUM (gates^T layout: gate-dim on partitions).
  - LSTM recurrence: per step, 16 weight-stationary bf16 matmuls accumulate
    W_hh @ h_{t-1} on top of the preloaded input projection in PSUM; gate
    nonlinearities on ScalarE, cell/hidden update on VectorE.
  - h history written to DRAM fp32 in its native on-chip layout
    [chunk, h-dim, (t, half, b)]; the host de-permutes while unsharding.
"""

import os
import sys

os.environ.setdefault("TILE_EXHAUSTIVE_MEMORY_SHARE_CHECK", "1")
sys.path.insert(0, "/opt/trn_rl_repo")

import numpy as np
import ml_dtypes

import concourse.tile as tile
from concourse import bacc, mybir, library_config
from concourse import bass_utils

BF16 = mybir.dt.bfloat16
F32 = mybir.dt.float32
I16 = mybir.dt.int16

V, E, H = 32000, 256, 256
G4 = 4 * H  # 1024 gate dims, 8 m-tiles of 128
NCORES = 8
BL = 8  # batch per core
S_FULL = 2048
PCHUNK = 32  # steps per PSUM chunk (4 banks)
GS = 64  # steps per gather chunk (dma_gather breaks somewhere in 512<n_idx<=1024)

# psum m-position gate map after host-side row permutation: i=(0,1) f=(2,3)
# o=(4,5) g=(6,7).  Emission order: g first (its Tanh overlaps the MM phase),
# o last (the merged Sigmoid over i,f,o starts when the phase ends).
MM_ORDER = [6, 7, 0, 1, 2, 3, 4, 5]


def build_nc(S=S_FULL, gs=GS):
    NP = S // PCHUNK
    NG = S // gs
    NIDX = gs * BL  # indices per gather chunk
    SPG = gs // PCHUNK  # pchunks per gather chunk

    nc = bacc.Bacc("TRN2", target_bir_lowering=False, debug=False)

    idx_d = nc.dram_tensor("idx", [NG, 128, NIDX // 16], I16, kind="ExternalInput")
    emb_d = nc.dram_tensor("embed", [V, E], BF16, kind="ExternalInput")
    wih_d = nc.dram_tensor("wih", [2, 128, G4], BF16, kind="ExternalInput")
    whh_d = nc.dram_tensor("whh", [2, 128, G4], BF16, kind="ExternalInput")
    bias_d = nc.dram_tensor("bias", [1, G4], BF16, kind="ExternalInput")
    out_d = nc.dram_tensor("out", [S // PCHUNK, 128, PCHUNK * 16], F32, kind="ExternalOutput")

    with tile.TileContext(nc) as tc:
        wpool = tc.alloc_tile_pool(name="w", bufs=1)
        ipool = tc.alloc_tile_pool(name="ip", bufs=2)
        xpool = tc.alloc_tile_pool(name="xp", bufs=2)
        pspool = tc.alloc_tile_pool(name="ps", bufs=2, space="PSUM")
        hpool = tc.alloc_tile_pool(name="hp", bufs=3)
        cpool = tc.alloc_tile_pool(name="cp", bufs=3)
        tpool = tc.alloc_tile_pool(name="tp", bufs=4)
        histpool = tc.alloc_tile_pool(name="hist", bufs=2)

        nc.gpsimd.load_library(library_config.mlp)

        # --- weights to SBUF ---
        wih = wpool.tile([128, 2, G4], BF16)
        nc.sync.dma_start(out=wih[:, :, :], in_=wih_d.ap().rearrange("c p n -> p c n"))
        whh = wpool.tile([128, 2, G4], BF16)
        nc.sync.dma_start(out=whh[:, :, :], in_=whh_d.ap().rearrange("c p n -> p c n"))
        bias = wpool.tile([1, G4], BF16)
        nc.sync.dma_start(out=bias[:, :], in_=bias_d[:, :])
        ones = wpool.tile([1, 256], BF16)
        nc.vector.memset(ones[:, :], 1.0)

        # --- initial state ---
        hT = hpool.tile([128, 2 * BL], BF16, tag="h")
        nc.vector.memset(hT[:, :], 0.0)
        cS0 = cpool.tile([128, 2 * BL], F32, tag="c")
        nc.vector.memset(cS0[:, :], 0.0)
        cS = cS0[:, :].rearrange("p (c b) -> p c b", c=2)

        xts = [None] * NG
        pss = [None] * NP

        def emit_gather(g):
            idx_sb = ipool.tile([128, NIDX // 16], I16)
            nc.sync.dma_start(out=idx_sb[:, :], in_=idx_d[g, :, :])
            xts[g] = xpool.tile([128, 2, NIDX], BF16, tag="xt", name="xt")
            nc.gpsimd.dma_gather(
                xts[g][:, :, :],
                emb_d[:, :],
                idx_sb[:, :],
                NIDX,
                NIDX,
                E,
                transpose=True,
            )

        # preGEMM piece list: per bank, even-m c0 (start=True) first
        PIECES = []
        for bk in range(4):
            me, mo = 2 * bk, 2 * bk + 1
            PIECES += [(me, 0, True), (mo, 0, False), (me, 1, False), (mo, 1, False)]
        PIECES += [(m, "bias", False) for m in range(8)]

        def emit_pregemm_piece(pc, i):
            if i == 0:
                pss[pc] = pspool.tile([128, PCHUNK * 64], F32, tag="psc", name="psc")
            ps = pss[pc]
            g, t0 = (pc * PCHUNK) // gs, (pc * PCHUNK) % gs
            m, c, st = PIECES[i]
            if c == "bias":
                return nc.tensor.matmul(
                    ps[:, m * 256 : (m + 1) * 256],
                    bias[0:1, m * 128 : (m + 1) * 128],
                    ones[0:1, 0:256],
                    start=False,
                    stop=False,
                    skip_group_check=True,
                )
            else:
                return nc.tensor.matmul(
                    ps[:, m * 256 : (m + 1) * 256],
                    wih[:, c, m * 128 : (m + 1) * 128],
                    xts[g][:, c, t0 * BL : (t0 + PCHUNK) * BL],
                    start=st,
                    stop=False,
                    skip_group_check=True,
                )

        def emit_step(t, hist):
            nonlocal hT, cS
            pc, tl = t // PCHUNK, t % PCHUNK
            ps = pss[pc]
            for c2 in range(2):
                for m in MM_ORDER:
                    nc.tensor.matmul(
                        ps[:, m * 256 + tl * BL : m * 256 + (tl + 1) * BL],
                        whh[:, c2, m * 128 : (m + 1) * 128],
                        hT[:, c2 * BL : (c2 + 1) * BL],
                        start=False,
                        stop=(m == 5 and c2 == 1),
                        skip_group_check=True,
                    )
            psv = ps[:, :].rearrange("p (m t b) -> p m t b", m=8, t=PCHUNK)
            tg = tpool.tile([128, 2 * BL], F32, tag="tg")
            nc.scalar.activation(
                tg[:, :].rearrange("p (c b) -> p c b", c=2),
                psv[:, 6:8, tl, :],
                mybir.ActivationFunctionType.Tanh,
            )
            sifo = tpool.tile([128, 6 * BL], F32, tag="sifo")
            nc.scalar.activation(
                sifo[:, :].rearrange("p (c b) -> p c b", c=6),
                psv[:, 0:6, tl, :],
                mybir.ActivationFunctionType.Sigmoid,
            )

            t1 = tpool.tile([128, 2 * BL], F32, tag="t1")
            nc.vector.tensor_mul(t1[:, :], sifo[:, 0 : 2 * BL], tg[:, :])
            t2 = tpool.tile([128, 2 * BL], F32, tag="t2")
            nc.vector.tensor_mul(
                t2[:, :].rearrange("p (c b) -> p c b", c=2),
                sifo[:, 2 * BL : 4 * BL].rearrange("p (c b) -> p c b", c=2),
                cS,
            )
            # store c in the dead g-gate psum slice of this step: Tanh then
            # reads PSUM (~75ns faster than an SBUF-source ACT op)
            cN = psv[:, 6:8, tl, :]
            cn_inst = nc.vector.tensor_add(
                cN,
                t1[:, :].rearrange("p (c b) -> p c b", c=2),
                t2[:, :].rearrange("p (c b) -> p c b", c=2),
            )
            tc_ = tpool.tile([128, 2 * BL], F32, tag="tc")
            nc.scalar.activation(
                tc_[:, :].rearrange("p (c b) -> p c b", c=2),
                cN,
                mybir.ActivationFunctionType.Tanh,
            )
            hN = hpool.tile([128, 2 * BL], BF16, tag="h")
            nc.vector.tensor_mul(hN[:, 0:BL], sifo[:, 4 * BL : 5 * BL], tc_[:, 0:BL])
            nc.vector.tensor_mul(hN[:, BL : 2 * BL], sifo[:, 5 * BL : 6 * BL], tc_[:, BL : 2 * BL])
            nc.vector.tensor_mul(
                hist[:, tl * 16 : (tl + 1) * 16], sifo[:, 4 * BL : 6 * BL], tc_[:, :]
            )
            hT, cS = hN, cN
            return cn_inst

        def emit_out(pc, hist):
            nc.sync.dma_start(out=out_d[pc, :, :], in_=hist[:, :])

        emit_gather(0)
        for i in range(len(PIECES)):
            emit_pregemm_piece(0, i)
        for pc in range(NP):
            if pc % SPG == 0 and pc // SPG + 1 < NG:
                emit_gather(pc // SPG + 1)
            hist = histpool.tile([128, PCHUNK * 16], F32)
            for s in range(PCHUNK):
                cn_inst = emit_step(pc * PCHUNK + s, hist)
                if pc + 1 < NP and s < len(PIECES):
                    pinst = emit_pregemm_piece(pc + 1, s)
                    tile.add_dep_helper(pinst.ins, cn_inst.ins, sync=True, reason="pe warmup")
            emit_out(pc, hist)

        for p in (histpool, tpool, cpool, hpool, pspool, xpool, ipool, wpool):
            p.release()

    nc.compile()
    return nc


def make_inputs(text_inputs, embed, W_ih, W_hh, b_ih, b_hh, S=S_FULL, gs=GS):
    """Host-side marshaling into per-core in_maps."""
    NG = S // gs
    NIDX = gs * BL
    tok = np.asarray(text_inputs).astype(np.int32)
    emb_bf = np.asarray(embed).astype(ml_dtypes.bfloat16)
    # permute gate rows [i, f, g, o] -> [i, f, o, g] (kernel's psum layout)
    perm = np.concatenate([np.arange(0, 512), np.arange(768, 1024), np.arange(512, 768)])
    W_ih = np.asarray(W_ih)[perm]
    W_hh = np.asarray(W_hh)[perm]
    bsum = (np.asarray(b_ih) + np.asarray(b_hh))[perm]
    wih_t = np.ascontiguousarray(W_ih.T).reshape(2, 128, G4).astype(ml_dtypes.bfloat16)
    whh_t = np.ascontiguousarray(W_hh.T).reshape(2, 128, G4).astype(ml_dtypes.bfloat16)
    bias = bsum.reshape(1, G4).astype(ml_dtypes.bfloat16)

    in_maps = []
    for m in range(NCORES):
        tc_ = tok[m * BL : (m + 1) * BL, :S]
        idx = np.empty((NG, 128, NIDX // 16), np.int16)
        for g in range(NG):
            flat = tc_[:, g * gs : (g + 1) * gs].T.reshape(-1)  # (t, b) order
            wrapped = flat.reshape(-1, 16).T.astype(np.int16)  # [16, NIDX//16]
            idx[g] = np.tile(wrapped, (8, 1))
        in_maps.append(
            {"idx": idx, "embed": emb_bf, "wih": wih_t, "whh": whh_t, "bias": bias}
        )
    return in_maps


def unpermute_out(raw):
    """[NP, 128, PCHUNK*16] (ch, p, (tl, c, b)) -> [BL, S, 256]"""
    NP = raw.shape[0]
    v = raw.reshape(NP, 128, PCHUNK, 2, BL)  # ch, p, tl, c, b
    v = v.transpose(4, 0, 2, 3, 1)  # b, ch, tl, c, p
    return np.ascontiguousarray(v).reshape(BL, NP * PCHUNK, 2 * 128)


_nc_cache = {}


def _get_nc(S=S_FULL, gs=GS):
    key = (S, gs)
    if key not in _nc_cache:
        _nc_cache[key] = build_nc(S, gs)
    return _nc_cache[key]


def kernel(text_inputs, mask_input, len_seq, embed, W_ih, W_hh, b_ih, b_hh):
    nc = _get_nc()
    in_maps = make_inputs(text_inputs, embed, W_ih, W_hh, b_ih, b_hh)
    try:
        res = bass_utils.run_bass_kernel_spmd(nc, in_maps, core_ids=list(range(NCORES)))
    except Exception:
        # transient device-state failures recover on retry
        res = bass_utils.run_bass_kernel_spmd(nc, in_maps, core_ids=list(range(NCORES)))
    out = np.concatenate(
        [unpermute_out(res.results[m]["out"]) for m in range(NCORES)], axis=0
    )
    mask = np.asarray(mask_input)
    if not np.all(mask == 1.0):
        out = out * mask[..., None]
    return out.astype(np.float32)
